# revision 20
# baseline (speedup 1.0000x reference)
"""DynamicPatchSelection TRN2 kernel: conv -> attn -> tanh -> bilinear patch sampling.

Data-parallel over batch: 16 images, 8 cores, 2 images/core. Weights replicated.
Host does layout marshalling only (padding, transposes, small weight folds);
all FLOPs over activations run on device.

Math notes:
  - conv via im2col (K = 3*9*9 = 243 + ones-row for bias), pixels-on-partitions
    so conv output chunks are directly the K-chunks of the tok projection.
  - tok/q/k/v in "transposed" layout (E on partitions, tokens on free dim) so
    all biases are per-partition ACT activation biases.
  - o2/out_w/fc collapse: logits = o @ (wo@out_w@fc_w) + const, because feat2
    only feeds the fc head (pure reassociation, folded on host in fp64).
  - sampling: ix_j = 32*tx + 27.5 + j  =>  per patch a 10x10 window at integer
    offsets with a single (wx, wy) blend. Gather via indirect DMA from an
    edge-padded (pad=6) image, one offset per patch per channel reading a
    760-float row-span; strided APs slice the 10x10 window out of it.
"""
import numpy as np

import concourse.bass as bass
import concourse.tile as tile
from concourse import bacc, mybir
from concourse.masks import make_identity

FP = mybir.dt.float32
I32 = mybir.dt.int32

N_CORES = 8
B, C, H, W = 16, 3, 64, 64
B2 = B // N_CORES          # images per core
P, PS, E, NH, POSD = 256, 9, 256, 8, 64
HW = H * W
DH = E // NH
PA = 4                     # conv zero-pad -> 72x72
WA = H + 2 * PA
PSD = 6                    # sampling edge-pad -> 76x76
WS = H + 2 * PSD           # 76
CHW_S = C * WS * WS        # 17328 floats per padded image
KTOT = C * PS * PS         # 243
K2N = KTOT - 128 + 1       # 116 rows in k-chunk 2 (115 weights + ones row)
NPIX = B2 * HW             # 8192 pixel columns per core
NCH = NPIX // 128          # 64 pixel chunks
SCALE = 1.0 / np.sqrt(DH)

_CACHE = {}


def _build():
    nc = bacc.Bacc("TRN2", target_bir_lowering=False, debug=False,
                   num_devices=N_CORES)

    def din(name, shape, dt=FP):
        return nc.dram_tensor(name, list(shape), dt, kind="ExternalInput").ap()

    def dout(name, shape, dt=FP):
        return nc.dram_tensor(name, list(shape), dt, kind="ExternalOutput").ap()

    xpadA = din("xpadA", (B2, C, WA, WA))
    xpadS = din("xpadS", (B2 * CHW_S, 1))
    onesrow = din("onesrow", (1, NPIX))
    convw = din("convw", (KTOT + 1, P))         # row 243 = conv_b
    in_w_d = din("in_w_d", (HW, E))
    in_b_d = din("in_b_d", (128, 2))
    wq_d = din("wq_d", (E, E)); bq_d = din("bq_d", (128, 2))
    wk_d = din("wk_d", (E, E)); bk_d = din("bk_d", (128, 2))
    wv_d = din("wv_d", (E, E)); bv_d = din("bv_d", (128, 2))
    w2_d = din("w2_d", (E, 2)); b2_d = din("b2_d", (2, 1))
    posw_d = din("posw_d", (3, POSD))           # row 2 = pos_b
    crow_d = din("crow_d", (128, 3))            # c*5776 channel base offsets

    patches_o = dout("patches_o", (B2, P, C * PS * PS))
    pos_o = dout("pos_o", (B2, P, POSD))

    from contextlib import ExitStack
    with tile.TileContext(nc) as tc, ExitStack() as ctx:
        wpool = ctx.enter_context(tc.tile_pool(name="wpool", bufs=1))
        imcp = ctx.enter_context(tc.tile_pool(name="imcp", bufs=1))
        featp = ctx.enter_context(tc.tile_pool(name="featp", bufs=3))
        sbp = ctx.enter_context(tc.tile_pool(name="sbp", bufs=3))
        persist = ctx.enter_context(tc.tile_pool(name="persist", bufs=1))
        smallp = ctx.enter_context(tc.tile_pool(name="smallp", bufs=4))
        sampp = ctx.enter_context(tc.tile_pool(name="sampp", bufs=2))
        ps_mm = ctx.enter_context(tc.tile_pool(name="ps_mm", bufs=3, space="PSUM"))
        ps_acc = ctx.enter_context(tc.tile_pool(name="ps_acc", bufs=4, space="PSUM"))

        # ---------------- weights into SBUF ----------------
        cw1 = wpool.tile([128, P], FP, tag="cw1")
        nc.sync.dma_start(cw1[:], convw[0:128, :])
        cw2 = wpool.tile([K2N, P], FP, tag="cw2")
        nc.sync.dma_start(cw2[0:K2N - 1, :], convw[128:KTOT, :])
        nc.sync.dma_start(cw2[K2N - 1:K2N, :], convw[KTOT:KTOT + 1, :])

        # in_w: 32 K-chunks of (128, 256) side by side -> (128, 8192)
        inw = wpool.tile([128, 32 * E], FP, tag="inw")
        nc.sync.dma_start(
            inw[:],
            bass.AP(tensor=in_w_d.tensor, offset=0,
                    ap=[[E, 128], [128 * E, 32], [1, E]]))

        def load_proj(w_d, tag):
            t = wpool.tile([128, 2 * E], FP, tag=tag)
            nc.sync.dma_start(
                t[:],
                bass.AP(tensor=w_d.tensor, offset=0,
                        ap=[[E, 128], [128 * E, 2], [1, E]]))
            return t

        wq = load_proj(wq_d, "wq")
        wk = load_proj(wk_d, "wk")
        wv = load_proj(wv_d, "wv")

        w2 = wpool.tile([128, 4], FP, tag="w2")
        nc.sync.dma_start(
            w2[:],
            bass.AP(tensor=w2_d.tensor, offset=0,
                    ap=[[2, 128], [128 * 2, 2], [1, 2]]))

        inb = wpool.tile([128, 2], FP, tag="inb")
        nc.sync.dma_start(inb[:], in_b_d[:])
        bq = wpool.tile([128, 2], FP, tag="bq"); nc.sync.dma_start(bq[:], bq_d[:])
        bk = wpool.tile([128, 2], FP, tag="bk"); nc.sync.dma_start(bk[:], bk_d[:])
        bv = wpool.tile([128, 2], FP, tag="bv"); nc.sync.dma_start(bv[:], bv_d[:])
        b2t = wpool.tile([2, 1], FP, tag="b2t"); nc.sync.dma_start(b2t[:], b2_d[:])
        posw = wpool.tile([3, POSD], FP, tag="posw")
        nc.sync.dma_start(posw[:], posw_d[:])
        crow = wpool.tile([128, 3], FP, tag="crow")
        nc.sync.dma_start(crow[:], crow_d[:])

        ident = wpool.tile([128, 128], FP, tag="ident")
        make_identity(nc, ident[:])

        # ---------------- im2col ----------------
        imc1 = imcp.tile([128, NPIX], FP, tag="imc1")
        imc2 = imcp.tile([K2N, NPIX], FP, tag="imc2")
        nc.sync.dma_start(imc2[K2N - 1:K2N, :], onesrow[:])

        for c in range(C):
            for dy in range(PS):
                k0 = c * 81 + dy * 9
                segs = []
                if k0 + 9 <= 128:
                    segs.append((imc1, k0, 0, 9))
                elif k0 >= 128:
                    segs.append((imc2, k0 - 128, 0, 9))
                else:
                    n1 = 128 - k0
                    segs.append((imc1, k0, 0, n1))
                    segs.append((imc2, 0, n1, 9 - n1))
                for (dst, dk, dx0, ndx) in segs:
                    for b in range(B2):
                        src = bass.AP(
                            tensor=xpadA.tensor,
                            offset=b * C * WA * WA + c * WA * WA + dy * WA + dx0,
                            ap=[[1, ndx], [WA, H], [1, W]])
                        nc.sync.dma_start(
                            dst[dk:dk + ndx, b * HW:(b + 1) * HW], src)

        # ---------------- conv + tok ----------------
        tokT_ps = [[ps_acc.tile([128, P], FP, tag="tok", name="tokps") for _ in range(2)]
                   for _ in range(B2)]
        for i in range(NCH):
            b, kc = divmod(i, 32)
            pc = ps_mm.tile([128, P], FP, tag="mm")
            nc.tensor.matmul(pc[:], imc1[:, i * 128:(i + 1) * 128], cw1[:],
                             start=True, stop=False)
            nc.tensor.matmul(pc[:], imc2[:, i * 128:(i + 1) * 128], cw2[:],
                             start=False, stop=True)
            ft = featp.tile([128, P], FP, tag="ft")
            nc.scalar.activation(ft[:], pc[:], mybir.ActivationFunctionType.Relu)
            for mh in range(2):
                nc.tensor.matmul(
                    tokT_ps[b][mh][:],
                    inw[:, kc * E + mh * 128: kc * E + mh * 128 + 128],
                    ft[:], start=(kc == 0), stop=(kc == 31))

        tokT = [[persist.tile([128, P], FP, tag=f"tokT{b}{mh}", name=f"tokT{b}{mh}") for mh in range(2)]
                for b in range(B2)]
        for b in range(B2):
            for mh in range(2):
                nc.scalar.activation(
                    tokT[b][mh][:], tokT_ps[b][mh][:],
                    mybir.ActivationFunctionType.Identity,
                    bias=inb[:, mh:mh + 1])

        # ---------------- q/k/v projections ----------------
        def proj(w_t, b_t, tag):
            # 4 tiles of 64 partitions per image so each head starts at
            # partition 0 or 32 (matmul operands can't start at 96).
            outs = []
            for b in range(B2):
                row = []
                for mh in range(2):
                    pp = ps_mm.tile([128, P], FP, tag="mm")
                    for kc in range(2):
                        nc.tensor.matmul(
                            pp[:],
                            w_t[:, kc * E + mh * 128: kc * E + mh * 128 + 128],
                            tokT[b][kc][:], start=(kc == 0), stop=(kc == 1))
                    for hh in range(2):
                        st = persist.tile([64, P], FP, tag=f"{tag}{b}{mh}{hh}",
                                          name=f"{tag}{b}{mh}{hh}")
                        nc.scalar.activation(
                            st[:], pp[hh * 64:(hh + 1) * 64, :],
                            mybir.ActivationFunctionType.Identity,
                            bias=b_t[hh * 64:(hh + 1) * 64, mh:mh + 1])
                        row.append(st)
                outs.append(row)
            return outs

        qT = proj(wq, bq, "qT")
        kT = proj(wk, bk, "kT")
        vT = proj(wv, bv, "vT")

        # ---------------- attention ----------------
        oT = [[persist.tile([128, P], FP, tag=f"oT{b}{mh}", name=f"oT{b}{mh}") for mh in range(2)]
              for b in range(B2)]
        for b in range(B2):
            for h in range(NH):
                mh, pr = divmod(h, 2)
                pr *= DH
                omh, opr = divmod(h, 4)
                aT = [sbp.tile([128, P], FP, tag="aT", name="aT") for _ in range(2)]
                for qc in range(2):
                    ssum = smallp.tile([128, 1], FP, tag="ssum")
                    att = sbp.tile([128, P], FP, tag="att")
                    sc = ps_mm.tile([128, P], FP, tag="mm")
                    nc.tensor.matmul(
                        sc[:],
                        qT[b][mh][pr:pr + DH, qc * 128:(qc + 1) * 128],
                        kT[b][mh][pr:pr + DH, :], start=True, stop=True)
                    nc.scalar.activation(att[:], sc[:],
                                         mybir.ActivationFunctionType.Exp,
                                         scale=float(SCALE), accum_out=ssum[:])
                    rs = smallp.tile([128, 1], FP, tag="rs")
                    nc.vector.reciprocal(rs[:], ssum[:])
                    nc.vector.tensor_scalar_mul(att[:], att[:], rs[:])
                    for kc in range(2):
                        tp_ = ps_mm.tile([128, 128], FP, tag="mm")
                        nc.tensor.transpose(tp_[:], att[:, kc * 128:(kc + 1) * 128],
                                            ident[:])
                        nc.vector.tensor_copy(
                            aT[kc][:, qc * 128:(qc + 1) * 128], tp_[:])
                o_ps = ps_mm.tile([DH, P], FP, tag="mm")
                for kc in range(2):
                    vv_ps = ps_mm.tile([128, DH], FP, tag="mm")
                    nc.tensor.transpose(
                        vv_ps[:], vT[b][mh][pr:pr + DH, kc * 128:(kc + 1) * 128],
                        ident[pr:pr + DH, pr:pr + DH])
                    vv = smallp.tile([128, DH], FP, tag="vv")
                    nc.vector.tensor_copy(vv[:], vv_ps[:])
                    nc.tensor.matmul(o_ps[:], vv[:], aT[kc][:],
                                     start=(kc == 0), stop=(kc == 1))
                nc.scalar.copy(oT[b][omh][opr * DH:(opr + 1) * DH, :], o_ps[:])

        # ---------------- logits -> tanh -> tp ----------------
        tpT = [persist.tile([3, P], FP, tag=f"tpT{b}", name=f"tpT{b}") for b in range(B2)]
        for b in range(B2):
            lg = ps_mm.tile([2, P], FP, tag="mm")
            for kc in range(2):
                nc.tensor.matmul(lg[:], w2[:, kc * 2:(kc + 1) * 2], oT[b][kc][:],
                                 start=(kc == 0), stop=(kc == 1))
            nc.sync.dma_start(tpT[b][2:3, :], onesrow[0:1, 0:P])
            nc.scalar.activation(tpT[b][0:2, :], lg[:],
                                 mybir.ActivationFunctionType.Tanh,
                                 bias=b2t[:])

        # ---------------- pos embeds ----------------
        for b in range(B2):
            for qc in range(2):
                pe = ps_mm.tile([128, POSD], FP, tag="mm")
                nc.tensor.matmul(pe[:], tpT[b][:, qc * 128:(qc + 1) * 128],
                                 posw[:], start=True, stop=True)
                pes = sbp.tile([128, POSD], FP, tag="pes")
                nc.scalar.copy(pes[:], pe[:])
                nc.sync.dma_start(pos_o[b, qc * 128:(qc + 1) * 128, :], pes[:])

        # ---------------- sampling ----------------
        for b in range(B2):
            for qc in range(2):
                t_ps = ps_mm.tile([128, 2], FP, tag="mm")
                nc.tensor.transpose(t_ps[:], tpT[b][0:2, qc * 128:(qc + 1) * 128],
                                    ident[0:2, 0:2])
                txy = smallp.tile([128, 2], FP, tag="txy")
                nc.vector.tensor_copy(txy[:], t_ps[:])
                u = smallp.tile([128, 2], FP, tag="u")
                nc.vector.tensor_scalar(u[:], txy[:], 32.0, 27.5 + PSD,
                                        op0=mybir.AluOpType.mult,
                                        op1=mybir.AluOpType.add)
                uh = smallp.tile([128, 2], FP, tag="uh")
                nc.vector.tensor_scalar_add(uh[:], u[:], -0.5)
                fi = smallp.tile([128, 2], I32, tag="fi")
                nc.vector.tensor_copy(fi[:], uh[:])      # round-to-nearest
                ff = smallp.tile([128, 2], FP, tag="ff")
                nc.vector.tensor_copy(ff[:], fi[:])
                wfr = smallp.tile([128, 2], FP, tag="wfr")
                nc.vector.tensor_tensor(wfr[:], u[:], ff[:],
                                        op=mybir.AluOpType.subtract)
                wm1 = smallp.tile([128, 2], FP, tag="wm1")
                nc.vector.tensor_scalar(wm1[:], wfr[:], -1.0, 1.0,
                                        op0=mybir.AluOpType.mult,
                                        op1=mybir.AluOpType.add)
                # offsets: base = Fy*76 + Fx + b*CHW_S ; + crow (c*5776 + i*76)
                ob = smallp.tile([128, 1], FP, tag="ob")
                nc.vector.tensor_scalar(ob[:], ff[:, 1:2], float(WS),
                                        float(b * CHW_S),
                                        op0=mybir.AluOpType.mult,
                                        op1=mybir.AluOpType.add)
                ob2 = smallp.tile([128, 1], FP, tag="ob2")
                nc.vector.tensor_tensor(ob2[:], ob[:], ff[:, 0:1],
                                        op=mybir.AluOpType.add)
                offf = smallp.tile([128, 3], FP, tag="offf")
                nc.vector.tensor_tensor(offf[:], crow[:],
                                        ob2[:].to_broadcast([128, 3]),
                                        op=mybir.AluOpType.add)
                offi = smallp.tile([128, 3], I32, tag="offi")
                nc.vector.tensor_copy(offi[:], offf[:])
                wnd = sampp.tile([128, 3 * 760], FP, tag="wnd")
                for c in range(C):
                    nc.gpsimd.indirect_dma_start(
                        out=wnd[:, c * 760:(c + 1) * 760],
                        out_offset=None,
                        in_=xpadS[:],
                        in_offset=bass.IndirectOffsetOnAxis(
                            ap=offi[:, c:c + 1], axis=0))
                w4 = wnd[:].rearrange("p (c i x) -> p c i x", c=3, i=10)
                xa = sampp.tile([128, 270], FP, tag="xa")
                xb_ = sampp.tile([128, 270], FP, tag="xb")
                nc.vector.tensor_scalar_mul(
                    xa[:].rearrange("p (c i j) -> p c i j", c=3, i=10),
                    w4[:, :, :, 0:9], wm1[:, 0:1])
                nc.vector.tensor_scalar_mul(
                    xb_[:].rearrange("p (c i j) -> p c i j", c=3, i=10),
                    w4[:, :, :, 1:10], wfr[:, 0:1])
                nc.vector.tensor_tensor(xa[:], xa[:], xb_[:],
                                        op=mybir.AluOpType.add)
                a4 = xa[:].rearrange("p (c i j) -> p c i j", c=3, i=10)
                ya = sampp.tile([128, 243], FP, tag="ya")
                yb = sampp.tile([128, 243], FP, tag="yb")
                nc.vector.tensor_scalar_mul(
                    ya[:].rearrange("p (c i j) -> p c i j", c=3, i=9),
                    a4[:, :, 0:9, :], wm1[:, 1:2])
                nc.vector.tensor_scalar_mul(
                    yb[:].rearrange("p (c i j) -> p c i j", c=3, i=9),
                    a4[:, :, 1:10, :], wfr[:, 1:2])
                nc.vector.tensor_tensor(ya[:], ya[:], yb[:],
                                        op=mybir.AluOpType.add)
                nc.sync.dma_start(patches_o[b, qc * 128:(qc + 1) * 128, :], ya[:])

    nc.compile()
    return nc


def _prep_in_maps(inputs):
    x = np.asarray(inputs["x"], np.float32)
    conv_w = np.asarray(inputs["conv_w"], np.float32)
    conv_b = np.asarray(inputs["conv_b"], np.float32)
    in_w = np.asarray(inputs["in_w"], np.float32)
    in_b = np.asarray(inputs["in_b"], np.float32)
    wq = np.asarray(inputs["wq"], np.float32); bq = np.asarray(inputs["bq"], np.float32)
    wk = np.asarray(inputs["wk"], np.float32); bk = np.asarray(inputs["bk"], np.float32)
    wv = np.asarray(inputs["wv"], np.float32); bv = np.asarray(inputs["bv"], np.float32)
    wo = np.asarray(inputs["wo"], np.float64); bo = np.asarray(inputs["bo"], np.float64)
    out_w = np.asarray(inputs["out_w"], np.float64)
    out_b = np.asarray(inputs["out_b"], np.float64)
    fc_w = np.asarray(inputs["fc_w"], np.float64)
    fc_b = np.asarray(inputs["fc_b"], np.float64)
    pos_w = np.asarray(inputs["pos_w"], np.float32)
    pos_b = np.asarray(inputs["pos_b"], np.float32)

    of = out_w @ fc_w                      # (E, 2) fp64
    w2 = (wo @ of).astype(np.float32)      # (E, 2)
    b2 = (bo @ of + out_b @ fc_w + fc_b).astype(np.float32)[:, None]

    convw = np.concatenate(
        [conv_w.transpose(1, 2, 3, 0).reshape(KTOT, P), conv_b[None, :]], 0)
    posw = np.concatenate([pos_w, pos_b[None, :]], 0)
    crow = np.tile((np.arange(3) * (WS * WS)).reshape(1, 3), (128, 1)) \
        .astype(np.float32)

    shared = dict(
        onesrow=np.ones((1, NPIX), np.float32),
        convw=convw, in_w_d=in_w, in_b_d=in_b.reshape(2, 128).T.copy(),
        wq_d=wq, bq_d=bq.reshape(2, 128).T.copy(), wk_d=wk, bk_d=bk.reshape(2, 128).T.copy(),
        wv_d=wv, bv_d=bv.reshape(2, 128).T.copy(), w2_d=w2, b2_d=b2,
        posw_d=posw, crow_d=crow)

    in_maps = []
    for ci in range(N_CORES):
        xb = x[ci * B2:(ci + 1) * B2]
        xa = np.pad(xb, ((0, 0), (0, 0), (PA, PA), (PA, PA)))
        xs = np.pad(xb, ((0, 0), (0, 0), (PSD, PSD), (PSD, PSD)), mode="edge")
        in_maps.append(dict(shared, xpadA=np.ascontiguousarray(xa),
                            xpadS=xs.reshape(-1, 1)))
    return in_maps


def _get_runner():
    """Build the Bass program once and return a cached jitted SPMD callable.

    Mirrors concourse.bass2jax.run_bass_via_pjrt's multi-core path, but keeps
    the jitted function across calls so repeat invocations don't recompile.
    """
    if "runner" in _CACHE:
        return _CACHE["runner"]
    import jax
    from jax.sharding import Mesh, PartitionSpec
    from jax.experimental.shard_map import shard_map
    import concourse.mybir as mb
    from concourse.bass2jax import (_bass_exec_p, install_neuronx_cc_hook,
                                    partition_id_tensor)

    nc = _build()
    install_neuronx_cc_hook()

    partition_name = (nc.partition_id_tensor.name
                      if nc.partition_id_tensor else None)
    in_names, out_names, out_avals, zero_outs = [], [], [], []
    for alloc in nc.m.functions[0].allocations:
        if not isinstance(alloc, mb.MemoryLocationSet):
            continue
        name = alloc.memorylocations[0].name
        if alloc.kind == "ExternalInput":
            if name != partition_name:
                in_names.append(name)
        elif alloc.kind == "ExternalOutput":
            out_names.append(name)
            shape = tuple(alloc.tensor_shape)
            dtype = mb.dt.np(alloc.dtype)
            out_avals.append(jax.core.ShapedArray(shape, dtype))
            zero_outs.append(np.zeros(shape, dtype))
    n_params = len(in_names)
    all_names = in_names + out_names
    if partition_name is not None:
        all_names = all_names + [partition_name]

    def _body(*args):
        operands = list(args)
        if partition_name is not None:
            operands.append(partition_id_tensor())
        outs = _bass_exec_p.bind(
            *operands,
            out_avals=tuple(out_avals),
            in_names=tuple(all_names),
            out_names=tuple(out_names),
            lowering_input_output_aliases=(),
            sim_require_finite=True,
            sim_require_nnan=True,
            nc=nc,
        )
        return tuple(outs)

    devices = jax.devices()[:N_CORES]
    mesh = Mesh(np.asarray(devices), ("core",))
    n_outs = len(out_names)
    in_specs = (PartitionSpec("core"),) * (n_params + n_outs)
    out_specs = (PartitionSpec("core"),) * n_outs
    donate = tuple(range(n_params, n_params + n_outs))
    sharded = jax.jit(
        shard_map(_body, mesh=mesh, in_specs=in_specs, out_specs=out_specs,
                  check_rep=False),
        donate_argnums=donate, keep_unused=True)

    runner = dict(fn=sharded, in_names=in_names, out_names=out_names,
                  out_avals=out_avals, zero_outs=zero_outs)
    _CACHE["runner"] = runner
    return runner


def _run(in_maps):
    r = _get_runner()
    concat_in = [np.concatenate([np.asarray(m[nm]) for m in in_maps], 0)
                 for nm in r["in_names"]]
    concat_zeros = [np.zeros((N_CORES * z.shape[0], *z.shape[1:]), z.dtype)
                    for z in r["zero_outs"]]
    out = r["fn"](*concat_in, *concat_zeros)
    return {nm: np.asarray(out[i]) for i, nm in enumerate(r["out_names"])}


def kernel(**inputs):
    in_maps = _prep_in_maps(inputs)
    out = _run(in_maps)
    patches = out["patches_o"].reshape(B, P, C * PS * PS)
    pos = out["pos_o"].reshape(B, P, POSD)
    return patches, pos
